# revision 23
# baseline (speedup 1.0000x reference)
"""Trainium2 Bass kernel for CrossAttention (B=2, N=M=2048, 16 heads x 64).

Sharding: batch x head-group parallel over 8 cores. Core c handles batch
c//4 and heads [4*(c%4), 4*(c%4)+4). Projection weights are column-split
(Wq/Wk/Wv) / row-split (Wo) per core; each core produces a partial
[2048, 1024] output (bf16) which the host sums per batch (4 partials).

V3 design (cost-model driven, all bf16):
  - Matmul cost = out_free_rows x cycles; contraction dim and output
    partition count are free.  The attn@V matmul therefore runs in
    "layout B": out[n 128, d 65] with lhsT = es[m, n-slice], rhs =
    v[m, 65] -- 66560 rows instead of 131072 (layout A).  Total PE:
    QKV proj 98304 + S 131072 + O 66560 + transpose 4096 + out-proj
    32768 = 332800 rows (138.7us floor at 2.4GHz).
  - es (exp of logits) persists in SBUF bf16 for 2.5 generations
    ((qc, head-pair) chunks); O accumulation is nt-major: each
    (n-tile, head) PSUM accumulator runs its 16 m-chunk matmuls
    back-to-back, so only 2 o-banks are live (PSUM: s 4 + fill-unit 1
    + fill-group 1 + o 2 = 8 banks).
  - Normalization: ones column in V gives denominators in o col 64;
    DVE reciprocal [P,1] + per-partition tensor_scalar_mul; PE
    transpose (identity matmul) flips [n, ic] -> [ic, n] for the
    output projection.
  - ACT runs only the softmax Exp (128 x [128,1024] = 132.9us).
  - Emission scheduling: a priority-class work queue (KT/QT/V/O/FIN
    units) pumped per exp-slot with a cycle budget plus forced drains
    at dependency barriers keeps PE saturated.
"""

import numpy as np
import ml_dtypes
from collections import deque
from contextlib import ExitStack

import concourse.tile as tile
from concourse import bacc, mybir
from concourse.bass_utils import run_bass_kernel_spmd

B, N, M, C = 2, 2048, 2048, 1024
HEADS, D = 16, 64
HPC = 4            # heads per core
IC = HPC * D       # 256 inner dims per core
SCALE = D ** -0.5
NCORES = 8
KT_TILES = C // 128   # 8 contraction tiles for projections
P = 128
MT = M // P           # 16 m tiles
QC = 512
NQC = N // QC         # 4 q chunks
NGEN = 2 * NQC        # 8 (qc, head-pair) generations
f32 = mybir.dt.float32
bf16 = mybir.dt.bfloat16

_CACHE = {}


def _body(nc, tc, ctx, xd, ctxd, wq, wk, wv, wo, identd, out, opt=None):
    opt = opt or {}
    SLOT_BUDGET = opt.get("slot_budget", 2500)
    HEAD_JUNK = opt.get("head_junk", 6)
    MID_JUNK = opt.get("mid_junk", 2)
    ES_BUFS = opt.get("es_bufs", 40)

    const = ctx.enter_context(tc.tile_pool(name="const", bufs=1))
    wq_sb = const.tile([P, KT_TILES, IC], bf16, tag="wq")
    wk_sb = const.tile([P, KT_TILES, IC], bf16, tag="wk")
    wv_sb = const.tile([P, KT_TILES, IC], bf16, tag="wv")
    wo_sb = const.tile([P, 2, C], bf16, tag="wo")
    ctx_sb = const.tile([P, KT_TILES, M], bf16, tag="ctx")
    x_sb = const.tile([P, KT_TILES, N], bf16, tag="x")
    kt_sb = [const.tile([P, M], bf16, tag=f"kt{j}", name=f"kt{j}") for j in range(2)]
    qt_sb = [const.tile([P, N], bf16, tag=f"qt{j}", name=f"qt{j}") for j in range(2)]
    v_sb = const.tile([P, MT, HPC, D + 1], bf16, tag="v")
    ao_sb = [const.tile([P, N], bf16, tag=f"ao{j}", name=f"ao{j}") for j in range(2)]
    ident_sb = const.tile([P, P], bf16, tag="ident")
    ones_sb = const.tile([P, 1], f32, tag="ones")
    junk_sb = const.tile([P, QC], bf16, tag="junk")

    es_pool = ctx.enter_context(tc.tile_pool(name="es", bufs=ES_BUFS))
    norm_pool = ctx.enter_context(tc.tile_pool(name="norm", bufs=2))
    r_pool = ctx.enter_context(tc.tile_pool(name="rp", bufs=4))
    outst_pool = ctx.enter_context(tc.tile_pool(name="outst", bufs=4))

    spool = ctx.enter_context(tc.tile_pool(name="s_ps", bufs=2, space="PSUM"))
    fu = ctx.enter_context(tc.tile_pool(name="fu_ps", bufs=1, space="PSUM"))
    fg = ctx.enter_context(tc.tile_pool(name="fg_ps", bufs=1, space="PSUM"))
    opool = ctx.enter_context(tc.tile_pool(name="o_ps", bufs=2, space="PSUM"))

    # junk tile first on DVE so PE warmup matmuls can start ~0.4us in
    nc.vector.memset(junk_sb[:], 0.0)
    nc.vector.memset(ones_sb[:], 1.0)
    # ones column of V_aug (denominator trick)
    nc.vector.tensor_copy(
        v_sb[:, :, :, D:D + 1],
        ones_sb[:, 0:1].to_broadcast((P, MT, HPC, 1)),
    )

    # ---- input DMAs ----
    # ACT must stay nearly DMA-free (exp starts ~8us and dma_start
    # occupies the issuing engine queue for the whole transfer): scalar
    # gets only wk; wq/wv ride the otherwise-idle DVE queue; x qc0
    # goes FIRST on SP/Pool (it gates the head QT -> first exp).
    nc.scalar.dma_start(wq_sb[:], wq[:])
    nc.scalar.dma_start(wk_sb[:], wk[:])
    for k in range(KT_TILES):
        eng = nc.sync if k < 4 else nc.gpsimd
        eng.dma_start(x_sb[:, k, 0:QC], xd[:, k, 0:QC])
    for k in range(KT_TILES):
        eng = nc.sync if k < 4 else nc.gpsimd
        eng.dma_start(ctx_sb[:, k, 0:QC], ctxd[:, k, 0:QC])
    for k in range(KT_TILES):
        eng = nc.sync if k % 2 == 0 else nc.gpsimd
        eng.dma_start(ctx_sb[:, k, QC:M], ctxd[:, k, QC:M])
    nc.sync.dma_start(wv_sb[:], wv[:])
    nc.gpsimd.dma_start(ident_sb[:], identd[:])
    nc.gpsimd.dma_start(wo_sb[:], wo[:])
    for k in range(KT_TILES):
        eng = nc.sync if k % 2 == 0 else nc.gpsimd
        eng.dma_start(x_sb[:, k, QC:N], xd[:, k, QC:N])
    # preload the Exp activation table so exp(0) doesn't pay ATL
    nc.scalar.activation(ones_sb[:], ones_sb[:],
                         mybir.ActivationFunctionType.Exp)

    def emit_junk(n, tag_n):
        # PE keep-warm matmuls (nothing reads the result): bridge idle
        # windows so the pstate ramp does not reset.
        for i in range(n):
            jp = fu.tile([P, QC], f32, tag="f", name=f"junk_{tag_n}_{i}")
            nc.tensor.matmul(
                jp[:], junk_sb[0:P, 0:P], junk_sb[:, 0:QC],
                start=True, stop=True,
            )

    # ---- fill unit emitters ----
    def kt_unit(p2, mt2, pool, tg):
        kp = pool.tile([P, QC], f32, tag=tg, name=f"kp{p2}_{mt2}")
        for k in range(KT_TILES):
            nc.tensor.matmul(
                kp[:, 0:P],
                wk_sb[:, k, p2 * P:(p2 + 1) * P],
                ctx_sb[:, k, mt2 * P:(mt2 + 1) * P],
                start=(k == 0), stop=(k == KT_TILES - 1),
            )
        nc.vector.tensor_copy(kt_sb[p2][:, mt2 * P:(mt2 + 1) * P], kp[:, 0:P])

    def v_unit(half, mt2, pool, tg):
        vt = pool.tile([P, QC], f32, tag=tg, name=f"vt{half}_{mt2}")
        for k in range(KT_TILES):
            nc.tensor.matmul(
                vt[:, 0:P],
                ctx_sb[:, k, mt2 * P:(mt2 + 1) * P],
                wv_sb[:, k, half * P:(half + 1) * P],
                start=(k == 0), stop=(k == KT_TILES - 1),
            )
        nc.vector.tensor_copy(
            v_sb[:, mt2, 2 * half:2 * half + 2, 0:D],
            vt[:, 0:P].rearrange("p (h d) -> p h d", d=D),
        )

    qt_state = {}

    def qt_item(qc2, j, k):
        if k == 0:
            qt_state[(qc2, j)] = fg.tile([P, QC], f32, tag="g",
                                         name=f"qg{qc2}_{j}")
        t = qt_state[(qc2, j)]
        nc.tensor.matmul(
            t[:],
            wq_sb[:, k, j * P:(j + 1) * P],
            x_sb[:, k, qc2 * QC:(qc2 + 1) * QC],
            start=(k == 0), stop=(k == KT_TILES - 1),
        )
        if k == KT_TILES - 1:
            nc.vector.tensor_copy(qt_sb[j][:, qc2 * QC:(qc2 + 1) * QC], t[:])
            del qt_state[(qc2, j)]

    def fin_unit(nt_g, ec, ft=None, stage_act=False):
        if ft is None:
            ft = fu.tile([P, QC], f32, tag="f", name=f"fin{nt_g}_{ec}")
        for j in range(2):
            nc.tensor.matmul(
                ft[:],
                ao_sb[j][:, nt_g * P:(nt_g + 1) * P],
                wo_sb[:, j, ec * QC:(ec + 1) * QC],
                start=(j == 0), stop=(j == 1),
            )
        ost = outst_pool.tile([P, QC], bf16, tag="ostg", name=f"og{nt_g}_{ec}")
        if stage_act:
            nc.scalar.copy(ost[:], ft[:])
        else:
            nc.vector.tensor_copy(ost[:], ft[:])
        deng = nc.sync if (nt_g + ec) % 2 == 0 else nc.gpsimd
        deng.dma_start(out[nt_g * P:(nt_g + 1) * P, ec * QC:(ec + 1) * QC],
                       ost[:])

    # ---- attention O-chunk machinery (layout B, nt-major) ----
    es_tiles = {}
    o_state = {}
    norm_state = {}

    def o_drain(g2, c):
        qc2, p2 = divmod(g2, 2)
        nt_l, hh = divmod(c, 2)
        nt_g = qc2 * 4 + nt_l
        ot = o_state.pop((g2, c))
        if hh == 0:
            norm_state[(g2, nt_l)] = norm_pool.tile(
                [P, P], bf16, tag="nm", name=f"nm{g2}_{nt_l}")
        nm = norm_state[(g2, nt_l)]
        r = r_pool.tile([P, 1], f32, tag="r", name=f"r{g2}_{c}")
        nc.vector.reciprocal(r[:], ot[:, D:D + 1])
        nc.vector.tensor_scalar_mul(nm[:, hh * D:(hh + 1) * D], ot[:, 0:D], r[:])
        if hh == 1:
            tp = fu.tile([P, P], bf16, tag="f", name=f"tp{g2}_{nt_l}")
            nc.tensor.transpose(tp[:], nm[:], ident_sb[:])
            nc.vector.tensor_copy(ao_sb[p2][:, nt_g * P:(nt_g + 1) * P], tp[:])
            del norm_state[(g2, nt_l)]

    def o_chunk(g2, c):
        qc2, p2 = divmod(g2, 2)
        nt_l, hh = divmod(c, 2)
        h = 2 * p2 + hh
        ot = opool.tile([P, D + 1], f32, tag="o", name=f"o{g2}_{c}")
        for mt2 in range(MT):
            nc.tensor.matmul(
                ot[:, 0:D + 1],
                es_tiles[(g2, mt2)][:, hh * QC + nt_l * P:
                                    hh * QC + (nt_l + 1) * P],
                v_sb[:, mt2, h, :],
                start=(mt2 == 0), stop=(mt2 == MT - 1),
            )
        o_state[(g2, c)] = ot
        if c >= 1:
            o_drain(g2, c - 1)

    # ---- priority-class work queue ----
    # item: (cls, idx, cyc, min_gen, fn)
    queue = deque()

    def seed():
        def add(cls, idx, cyc, min_gen, fn):
            queue.append((cls, idx, cyc, min_gen, fn))

        def add_kt(cls, p2, mts):
            for i, mt2 in enumerate(mts):
                add(cls, mt2, 1024, 0, (lambda m=mt2, i2=i, pp=p2: kt_unit(
                    pp, m, opool if i2 % 2 == 0 else fu,
                    "o" if i2 % 2 == 0 else "f")))

        def add_v(cls, half, alt):
            for i, mt2 in enumerate(range(MT)):
                pool, tg = (opool, "o") if (alt and i % 2 == 0) else (fu, "f")
                add(cls, mt2, 1024, 0,
                    (lambda m=mt2, h2=half, pl=pool, t2=tg: v_unit(h2, m, pl, t2)))

        def add_o(cls, g2):
            for c in range(8):
                add(cls, c, 1100, g2 + 1, (lambda g3=g2, c2=c: o_chunk(g3, c2)))
            add(cls, 8, 150, g2 + 1, (lambda g3=g2: o_drain(g3, 7)))

        def add_qt(cls, qc2, j):
            for k in range(KT_TILES):
                add(cls, k, QC, 0, (lambda q=qc2, j2=j, k2=k: qt_item(q, j2, k2)))

        def add_fin(cls, qc2):
            for nt_l in range(4):
                for ec in range(2):
                    add(cls, nt_l * 2 + ec, 1024, 0,
                        (lambda n=qc2 * 4 + nt_l, e=ec: fin_unit(n, e)))

        add_kt(0, 0, range(1, MT))
        add_qt(1, 0, 1)
        add_kt(2, 1, range(MT))
        add_qt(3, 1, 0)
        add_qt(4, 1, 1)
        add_v(5, 0, True)
        add_o(6, 0)
        add_v(7, 1, False)
        add_qt(8, 2, 0)
        add_qt(9, 2, 1)
        add_o(10, 1)
        add_o(11, 2)
        add_fin(12, 0)
        add_qt(13, 3, 0)
        add_qt(14, 3, 1)
        add_o(15, 3)
        add_fin(16, 1)
        add_o(17, 4)
        add_o(18, 5)
        add_fin(19, 2)
        add_o(20, 6)

    seed()
    QT_CLS = {(0, 1): 1, (1, 0): 3, (1, 1): 4, (2, 0): 8, (2, 1): 9,
              (3, 0): 13, (3, 1): 14}
    O_CLS = {0: 6, 1: 10, 2: 11, 3: 15, 4: 17, 5: 18, 6: 20}
    cur_gen = [0]

    def drain_thru(cls_id):
        spent = 0
        while queue and queue[0][0] <= cls_id:
            cls, idx, cyc, mg, fn = queue.popleft()
            assert mg <= cur_gen[0], f"forced drain of blocked item {cls}/{idx}"
            fn()
            spent += cyc
        return spent

    def drain_units(cls_id, max_idx):
        spent = 0
        while queue and (queue[0][0] < cls_id
                         or (queue[0][0] == cls_id and queue[0][1] <= max_idx)):
            cls, idx, cyc, mg, fn = queue.popleft()
            assert mg <= cur_gen[0]
            fn()
            spent += cyc
        return spent

    def pump(budget):
        spent = 0
        while queue and spent < budget and queue[0][3] <= cur_gen[0]:
            cls, idx, cyc, mg, fn = queue.popleft()
            fn()
            spent += cyc
        return spent

    # ---- head: warmup + QT(qc0, j0) (the longer pole: x DMA + 8 mm +
    # drain gate the first s) then KT m0, via separate s-pool tiles
    # (tile-granular deps would otherwise delay the qt drain) ----
    emit_junk(HEAD_JUNK, "h")
    h0 = spool.tile([P, 2 * QC], f32, tag="s", name="h0")
    for k in range(KT_TILES):
        nc.tensor.matmul(
            h0[:, 0:QC],
            wq_sb[:, k, 0:P],
            x_sb[:, k, 0:QC],
            start=(k == 0), stop=(k == KT_TILES - 1),
        )
    nc.vector.tensor_copy(qt_sb[0][:, 0:QC], h0[:, 0:QC])

    # ---- main loop: 8 generations x 16 exp slots ----
    # s matmuls are emitted ONE slot ahead of their exp so the exp
    # stream never eats the s-completion sem latency.
    s_tiles = {}

    def emit_s(g2, mt2):
        qc2, p2 = divmod(g2, 2)
        s_t = spool.tile([P, 2 * QC], f32, tag="s", name=f"s{g2}_{mt2}")
        for hh in range(2):
            nc.tensor.matmul(
                s_t[:, hh * QC:(hh + 1) * QC],
                kt_sb[p2][hh * D:(hh + 1) * D, mt2 * P:(mt2 + 1) * P],
                qt_sb[p2][hh * D:(hh + 1) * D, qc2 * QC:(qc2 + 1) * QC],
                start=True, stop=True,
            )
        s_tiles[(g2, mt2)] = s_t

    h1 = spool.tile([P, 2 * QC], f32, tag="s", name="h1")
    for k in range(KT_TILES):
        nc.tensor.matmul(
            h1[:, 0:P],
            wk_sb[:, k, 0:P],
            ctx_sb[:, k, 0:P],
            start=(k == 0), stop=(k == KT_TILES - 1),
        )
    nc.scalar.copy(kt_sb[0][:, 0:P], h1[:, 0:P])
    emit_s(0, 0)
    for g in range(NGEN):
        cur_gen[0] = g
        qc, p = divmod(g, 2)
        for mt in range(MT):
            spent = 0
            if g == 0:
                spent += drain_units(0, min(mt + 1, MT - 1))
            elif g == 1:
                spent += drain_units(2, min(mt + 1, MT - 1))
            if g >= 2:
                # pace the O(g-2) chunks (es-buffer reuse gate at mt==8
                # of this gen): ~1 chunk per slot, done by slot 8.
                spent += drain_units(O_CLS[g - 2], mt)
            if g + 2 < NGEN and g + 1 >= 2 and mt >= 12:
                # pace next gen's QT group drain across slots 12-15
                spent += drain_units(QT_CLS[divmod(g + 1, 2)],
                                     2 * (mt - 12) + 1)
            es_t = es_pool.tile([P, 2 * QC], bf16, tag="es", name=f"es{g}_{mt}")
            nc.scalar.activation(
                es_t[:], s_tiles.pop((g, mt)),
                mybir.ActivationFunctionType.Exp, scale=SCALE,
            )
            es_tiles[(g, mt)] = es_t
            if mt < MT - 1:
                emit_s(g, mt + 1)
            elif g + 1 < NGEN:
                if g + 1 == 1:
                    spent += drain_units(2, 0)
                elif g + 1 >= 2:
                    spent += drain_thru(QT_CLS[divmod(g + 1, 2)])
                emit_s(g + 1, 0)
            spent += 2 * QC
            pump(SLOT_BUDGET - spent)

    # ---- tail: O(gen 7) + output projection for qc3 ----
    # All chunks first (PE runs back-to-back), then the fins: keeps
    # DVE round-trips off the in-order PE queue's critical path. Tail
    # stage copies go to ACT (idle after the last exp).
    cur_gen[0] = NGEN
    drain_thru(20)
    g7 = NGEN - 1

    def tail_fins(nt_l):
        # tail fins use the (now idle) s-pool banks; stage copies split
        # across ACT (ec0) and DVE (ec1), both idle after the last exp.
        tt = spool.tile([P, 2 * QC], f32, tag="s", name=f"tfin{nt_l}")
        fin_unit(12 + nt_l, 0, ft=tt[:, 0:QC], stage_act=True)
        fin_unit(12 + nt_l, 1, ft=tt[:, QC:2 * QC], stage_act=False)

    fins_after = {2: 0, 4: 1, 6: 2}
    for c in range(8):
        o_chunk(g7, c)
        if c in fins_after:
            tail_fins(fins_after[c])
    o_drain(g7, 7)
    tail_fins(3)


def _build(reps=1, opt=None):
    key = (reps, tuple(sorted((opt or {}).items())))
    if key in _CACHE:
        return _CACHE[key]
    nc = bacc.Bacc("TRN2", target_bir_lowering=False, debug=False)
    xd = nc.dram_tensor("xd", [P, KT_TILES, N], bf16, kind="ExternalInput")
    ctxd = nc.dram_tensor("ctxd", [P, KT_TILES, M], bf16, kind="ExternalInput")
    wq = nc.dram_tensor("wq", [P, KT_TILES, IC], bf16, kind="ExternalInput")
    wk = nc.dram_tensor("wk", [P, KT_TILES, IC], bf16, kind="ExternalInput")
    wv = nc.dram_tensor("wv", [P, KT_TILES, IC], bf16, kind="ExternalInput")
    wo = nc.dram_tensor("wo", [P, 2, C], bf16, kind="ExternalInput")
    identd = nc.dram_tensor("ident", [P, P], bf16, kind="ExternalInput")
    out = nc.dram_tensor("out", [N, C], bf16, kind="ExternalOutput")
    with tile.TileContext(nc) as tc:
        for _ in range(reps):
            with ExitStack() as ctx:
                _body(nc, tc, ctx, xd, ctxd, wq, wk, wv, wo, identd, out,
                      opt=opt)
    nc.compile()
    _CACHE[key] = nc
    return nc


def _to_tiled(a, inner):
    """[K*128, inner] f32 -> [128, K, inner] bf16 (partition-major tiling)."""
    k = a.shape[0] // P
    return np.ascontiguousarray(
        a.reshape(k, P, inner).transpose(1, 0, 2).astype(ml_dtypes.bfloat16)
    )


def _shard_inputs(x, context, Wq, Wk, Wv, Wo):
    ident = np.eye(P, dtype=ml_dtypes.bfloat16)
    in_maps = []
    for c in range(NCORES):
        b, g = divmod(c, NCORES // B)
        cols = slice(g * IC, (g + 1) * IC)
        in_maps.append({
            "xd": _to_tiled(np.ascontiguousarray(x[b].T), N),
            "ctxd": _to_tiled(np.ascontiguousarray(context[b].T), M),
            "wq": _to_tiled(np.ascontiguousarray(Wq[:, cols]), IC),
            "wk": _to_tiled(np.ascontiguousarray(Wk[:, cols]), IC),
            "wv": _to_tiled(np.ascontiguousarray(Wv[:, cols]), IC),
            "wo": _to_tiled(np.ascontiguousarray(Wo[cols, :]), C),
            "ident": ident,
        })
    return in_maps


def kernel(x, context, Wq, Wk, Wv, Wo, reps=1):
    x = np.asarray(x, dtype=np.float32)
    context = np.asarray(context, dtype=np.float32)
    Wq, Wk, Wv, Wo = (np.asarray(w, dtype=np.float32) for w in (Wq, Wk, Wv, Wo))
    nc = _build(reps)
    in_maps = _shard_inputs(x, context, Wq, Wk, Wv, Wo)
    res = run_bass_kernel_spmd(nc, in_maps, core_ids=list(range(NCORES)))
    gpb = NCORES // B
    out = np.zeros((B, N, C), dtype=np.float32)
    for c in range(NCORES):
        out[c // gpb] += np.asarray(res.results[c]["out"], dtype=np.float32)
    return out


# revision 24
# speedup vs baseline: 1.0430x; 1.0430x over previous
"""Trainium2 Bass kernel for CrossAttention (B=2, N=M=2048, 16 heads x 64).

Sharding: batch x head-group parallel over 8 cores. Core c handles batch
c//4 and heads [4*(c%4), 4*(c%4)+4). Projection weights are column-split
(Wq/Wk/Wv) / row-split (Wo) per core; each core produces a partial
[2048, 1024] output (bf16) which the host sums per batch (4 partials).

V3 design (cost-model driven, all bf16):
  - Matmul cost = out_free_rows x cycles; contraction dim and output
    partition count are free.  The attn@V matmul therefore runs in
    "layout B": out[n 128, d 65] with lhsT = es[m, n-slice], rhs =
    v[m, 65] -- 66560 rows instead of 131072 (layout A).  Total PE:
    QKV proj 98304 + S 131072 + O 66560 + transpose 4096 + out-proj
    32768 = 332800 rows (138.7us floor at 2.4GHz).
  - es (exp of logits) persists in SBUF bf16 for 2.5 generations
    ((qc, head-pair) chunks); O accumulation is nt-major: each
    (n-tile, head) PSUM accumulator runs its 16 m-chunk matmuls
    back-to-back, so only 2 o-banks are live (PSUM: s 4 + fill-unit 1
    + fill-group 1 + o 2 = 8 banks).
  - Normalization: ones column in V gives denominators in o col 64;
    DVE reciprocal [P,1] + per-partition tensor_scalar_mul; PE
    transpose (identity matmul) flips [n, ic] -> [ic, n] for the
    output projection.
  - ACT runs only the softmax Exp (128 x [128,1024] = 132.9us).
  - Emission scheduling: a priority-class work queue (KT/QT/V/O/FIN
    units) pumped per exp-slot with a cycle budget plus forced drains
    at dependency barriers keeps PE saturated.
"""

import numpy as np
import ml_dtypes
from collections import deque
from contextlib import ExitStack

import concourse.tile as tile
from concourse import bacc, mybir
from concourse.bass_utils import run_bass_kernel_spmd

B, N, M, C = 2, 2048, 2048, 1024
HEADS, D = 16, 64
HPC = 4            # heads per core
IC = HPC * D       # 256 inner dims per core
SCALE = D ** -0.5
NCORES = 8
KT_TILES = C // 128   # 8 contraction tiles for projections
P = 128
MT = M // P           # 16 m tiles
QC = 512
NQC = N // QC         # 4 q chunks
NGEN = 2 * NQC        # 8 (qc, head-pair) generations
f32 = mybir.dt.float32
bf16 = mybir.dt.bfloat16

_CACHE = {}


def _body(nc, tc, ctx, xd, ctxd, wq, wk, wv, wo, identd, out, opt=None):
    opt = opt or {}
    SLOT_BUDGET = opt.get("slot_budget", 2500)
    HEAD_JUNK = opt.get("head_junk", 6)
    MID_JUNK = opt.get("mid_junk", 2)
    ES_BUFS = opt.get("es_bufs", 40)

    const = ctx.enter_context(tc.tile_pool(name="const", bufs=1))
    wq_sb = const.tile([P, KT_TILES, IC], bf16, tag="wq")
    wk_sb = const.tile([P, KT_TILES, IC], bf16, tag="wk")
    wv_sb = const.tile([P, KT_TILES, IC], bf16, tag="wv")
    wo_sb = const.tile([P, 2, C], bf16, tag="wo")
    ctx_sb = const.tile([P, KT_TILES, M], bf16, tag="ctx")
    x_sb = const.tile([P, KT_TILES, N], bf16, tag="x")
    kt_sb = [const.tile([P, M], bf16, tag=f"kt{j}", name=f"kt{j}") for j in range(2)]
    qt_sb = [const.tile([P, N], bf16, tag=f"qt{j}", name=f"qt{j}") for j in range(2)]
    v_sb = const.tile([P, MT, HPC, D + 1], bf16, tag="v")
    ao_sb = [const.tile([P, N], bf16, tag=f"ao{j}", name=f"ao{j}") for j in range(2)]
    ident_sb = const.tile([P, P], bf16, tag="ident")
    ones_sb = const.tile([P, 1], f32, tag="ones")
    junk_sb = const.tile([P, QC], bf16, tag="junk")

    es_pool = ctx.enter_context(tc.tile_pool(name="es", bufs=ES_BUFS))
    norm_pool = ctx.enter_context(tc.tile_pool(name="norm", bufs=2))
    r_pool = ctx.enter_context(tc.tile_pool(name="rp", bufs=4))
    outst_pool = ctx.enter_context(tc.tile_pool(name="outst", bufs=4))

    spool = ctx.enter_context(tc.tile_pool(name="s_ps", bufs=2, space="PSUM"))
    fu = ctx.enter_context(tc.tile_pool(name="fu_ps", bufs=1, space="PSUM"))
    fg = ctx.enter_context(tc.tile_pool(name="fg_ps", bufs=1, space="PSUM"))
    opool = ctx.enter_context(tc.tile_pool(name="o_ps", bufs=2, space="PSUM"))

    # junk tile first on DVE so PE warmup matmuls can start ~0.4us in
    nc.vector.memset(junk_sb[:], 0.0)
    nc.vector.memset(ones_sb[:], 1.0)
    # ones column of V_aug (denominator trick)
    nc.vector.tensor_copy(
        v_sb[:, :, :, D:D + 1],
        ones_sb[:, 0:1].to_broadcast((P, MT, HPC, 1)),
    )

    # ---- input DMAs ----
    # ACT must stay nearly DMA-free (exp starts ~8us and dma_start
    # occupies the issuing engine queue for the whole transfer): scalar
    # gets only wk; wq/wv ride the otherwise-idle DVE queue; x qc0
    # goes FIRST on SP/Pool (it gates the head QT -> first exp).
    nc.scalar.dma_start(wq_sb[:], wq[:])
    nc.scalar.dma_start(wk_sb[:], wk[:])
    for k in range(KT_TILES):
        eng = nc.sync if k < 4 else nc.gpsimd
        eng.dma_start(x_sb[:, k, 0:QC], xd[:, k, 0:QC])
    for k in range(KT_TILES):
        eng = nc.sync if k < 4 else nc.gpsimd
        eng.dma_start(ctx_sb[:, k, 0:QC], ctxd[:, k, 0:QC])
    for k in range(KT_TILES):
        eng = nc.sync if k % 2 == 0 else nc.gpsimd
        eng.dma_start(ctx_sb[:, k, QC:M], ctxd[:, k, QC:M])
    nc.sync.dma_start(wv_sb[:], wv[:])
    nc.gpsimd.dma_start(ident_sb[:], identd[:])
    nc.gpsimd.dma_start(wo_sb[:], wo[:])
    for k in range(KT_TILES):
        eng = nc.sync if k % 2 == 0 else nc.gpsimd
        eng.dma_start(x_sb[:, k, QC:N], xd[:, k, QC:N])
    # preload the Exp activation table so exp(0) doesn't pay ATL
    nc.scalar.activation(ones_sb[:], ones_sb[:],
                         mybir.ActivationFunctionType.Exp)

    def emit_junk(n, tag_n):
        # PE keep-warm matmuls (nothing reads the result): bridge idle
        # windows so the pstate ramp does not reset.
        for i in range(n):
            jp = fu.tile([P, QC], f32, tag="f", name=f"junk_{tag_n}_{i}")
            nc.tensor.matmul(
                jp[:], junk_sb[0:P, 0:P], junk_sb[:, 0:QC],
                start=True, stop=True,
            )

    # ---- fill unit emitters ----
    def kt_unit(p2, mt2, pool, tg):
        kp = pool.tile([P, QC], f32, tag=tg, name=f"kp{p2}_{mt2}")
        for k in range(KT_TILES):
            nc.tensor.matmul(
                kp[:, 0:P],
                wk_sb[:, k, p2 * P:(p2 + 1) * P],
                ctx_sb[:, k, mt2 * P:(mt2 + 1) * P],
                start=(k == 0), stop=(k == KT_TILES - 1),
            )
        nc.vector.tensor_copy(kt_sb[p2][:, mt2 * P:(mt2 + 1) * P], kp[:, 0:P])

    def v_unit(half, mt2, pool, tg):
        vt = pool.tile([P, QC], f32, tag=tg, name=f"vt{half}_{mt2}")
        for k in range(KT_TILES):
            nc.tensor.matmul(
                vt[:, 0:P],
                ctx_sb[:, k, mt2 * P:(mt2 + 1) * P],
                wv_sb[:, k, half * P:(half + 1) * P],
                start=(k == 0), stop=(k == KT_TILES - 1),
            )
        nc.vector.tensor_copy(
            v_sb[:, mt2, 2 * half:2 * half + 2, 0:D],
            vt[:, 0:P].rearrange("p (h d) -> p h d", d=D),
        )

    qt_state = {}

    def qt_item(qc2, j, k):
        if k == 0:
            qt_state[(qc2, j)] = fg.tile([P, QC], f32, tag="g",
                                         name=f"qg{qc2}_{j}")
        t = qt_state[(qc2, j)]
        nc.tensor.matmul(
            t[:],
            wq_sb[:, k, j * P:(j + 1) * P],
            x_sb[:, k, qc2 * QC:(qc2 + 1) * QC],
            start=(k == 0), stop=(k == KT_TILES - 1),
        )
        if k == KT_TILES - 1:
            nc.vector.tensor_copy(qt_sb[j][:, qc2 * QC:(qc2 + 1) * QC], t[:])
            del qt_state[(qc2, j)]

    def fin_unit(nt_g, ec, ft=None, stage_act=False):
        if ft is None:
            ft = fu.tile([P, QC], f32, tag="f", name=f"fin{nt_g}_{ec}")
        for j in range(2):
            nc.tensor.matmul(
                ft[:],
                ao_sb[j][:, nt_g * P:(nt_g + 1) * P],
                wo_sb[:, j, ec * QC:(ec + 1) * QC],
                start=(j == 0), stop=(j == 1),
            )
        ost = outst_pool.tile([P, QC], bf16, tag="ostg", name=f"og{nt_g}_{ec}")
        if stage_act:
            nc.scalar.copy(ost[:], ft[:])
        else:
            nc.vector.tensor_copy(ost[:], ft[:])
        deng = nc.sync if (nt_g + ec) % 2 == 0 else nc.gpsimd
        deng.dma_start(out[nt_g * P:(nt_g + 1) * P, ec * QC:(ec + 1) * QC],
                       ost[:])

    # ---- attention O-chunk machinery (layout B, nt-major) ----
    es_tiles = {}
    o_state = {}
    norm_state = {}

    def o_drain(g2, c):
        qc2, p2 = divmod(g2, 2)
        nt_l, hh = divmod(c, 2)
        nt_g = qc2 * 4 + nt_l
        ot = o_state.pop((g2, c))
        if hh == 0:
            norm_state[(g2, nt_l)] = norm_pool.tile(
                [P, P], bf16, tag="nm", name=f"nm{g2}_{nt_l}")
        nm = norm_state[(g2, nt_l)]
        r = r_pool.tile([P, 1], f32, tag="r", name=f"r{g2}_{c}")
        nc.vector.reciprocal(r[:], ot[:, D:D + 1])
        nc.vector.tensor_scalar_mul(nm[:, hh * D:(hh + 1) * D], ot[:, 0:D], r[:])
        if hh == 1:
            tp = fu.tile([P, P], bf16, tag="f", name=f"tp{g2}_{nt_l}")
            nc.tensor.transpose(tp[:], nm[:], ident_sb[:])
            nc.vector.tensor_copy(ao_sb[p2][:, nt_g * P:(nt_g + 1) * P], tp[:])
            del norm_state[(g2, nt_l)]

    def o_chunk(g2, c):
        qc2, p2 = divmod(g2, 2)
        nt_l, hh = divmod(c, 2)
        h = 2 * p2 + hh
        ot = opool.tile([P, D + 1], f32, tag="o", name=f"o{g2}_{c}")
        for mt2 in range(MT):
            nc.tensor.matmul(
                ot[:, 0:D + 1],
                es_tiles[(g2, mt2)][:, hh * QC + nt_l * P:
                                    hh * QC + (nt_l + 1) * P],
                v_sb[:, mt2, h, :],
                start=(mt2 == 0), stop=(mt2 == MT - 1),
            )
        o_state[(g2, c)] = ot
        if c >= 1:
            o_drain(g2, c - 1)

    # ---- priority-class work queue ----
    # item: (cls, idx, cyc, min_gen, fn)
    queue = deque()

    def seed():
        def add(cls, idx, cyc, min_gen, fn):
            queue.append((cls, idx, cyc, min_gen, fn))

        def add_kt(cls, p2, mts):
            for i, mt2 in enumerate(mts):
                add(cls, mt2, 1024, 0, (lambda m=mt2, i2=i, pp=p2: kt_unit(
                    pp, m, opool if i2 % 2 == 0 else fu,
                    "o" if i2 % 2 == 0 else "f")))

        def add_v(cls, half, alt):
            for i, mt2 in enumerate(range(MT)):
                pool, tg = (opool, "o") if (alt and i % 2 == 0) else (fu, "f")
                add(cls, mt2, 1024, 0,
                    (lambda m=mt2, h2=half, pl=pool, t2=tg: v_unit(h2, m, pl, t2)))

        def add_o(cls, g2):
            for c in range(8):
                add(cls, c, 1100, g2 + 1, (lambda g3=g2, c2=c: o_chunk(g3, c2)))
            add(cls, 8, 150, g2 + 1, (lambda g3=g2: o_drain(g3, 7)))

        def add_qt(cls, qc2, j):
            for k in range(KT_TILES):
                add(cls, k, QC, 0, (lambda q=qc2, j2=j, k2=k: qt_item(q, j2, k2)))

        def add_fin(cls, qc2):
            for nt_l in range(4):
                for ec in range(2):
                    add(cls, nt_l * 2 + ec, 1024, 0,
                        (lambda n=qc2 * 4 + nt_l, e=ec: fin_unit(n, e)))

        add_kt(0, 0, range(1, MT))
        add_qt(1, 0, 1)
        add_kt(2, 1, range(MT))
        add_qt(3, 1, 0)
        add_qt(4, 1, 1)
        add_v(5, 0, True)
        add_o(6, 0)
        add_v(7, 1, False)
        add_qt(8, 2, 0)
        add_qt(9, 2, 1)
        add_o(10, 1)
        add_o(11, 2)
        add_fin(12, 0)
        add_qt(13, 3, 0)
        add_qt(14, 3, 1)
        add_o(15, 3)
        add_fin(16, 1)
        add_o(17, 4)
        add_o(18, 5)
        add_fin(19, 2)
        add_o(20, 6)

    seed()
    QT_CLS = {(0, 1): 1, (1, 0): 3, (1, 1): 4, (2, 0): 8, (2, 1): 9,
              (3, 0): 13, (3, 1): 14}
    O_CLS = {0: 6, 1: 10, 2: 11, 3: 15, 4: 17, 5: 18, 6: 20}
    cur_gen = [0]

    def drain_thru(cls_id):
        spent = 0
        while queue and queue[0][0] <= cls_id:
            cls, idx, cyc, mg, fn = queue.popleft()
            assert mg <= cur_gen[0], f"forced drain of blocked item {cls}/{idx}"
            fn()
            spent += cyc
        return spent

    def drain_units(cls_id, max_idx):
        spent = 0
        while queue and (queue[0][0] < cls_id
                         or (queue[0][0] == cls_id and queue[0][1] <= max_idx)):
            cls, idx, cyc, mg, fn = queue.popleft()
            assert mg <= cur_gen[0]
            fn()
            spent += cyc
        return spent

    def pump(budget):
        spent = 0
        while queue and spent < budget and queue[0][3] <= cur_gen[0]:
            cls, idx, cyc, mg, fn = queue.popleft()
            fn()
            spent += cyc
        return spent

    # ---- head: warmup + QT(qc0, j0) (the longer pole: x DMA + 8 mm +
    # drain gate the first s) then KT m0, via separate s-pool tiles
    # (tile-granular deps would otherwise delay the qt drain) ----
    emit_junk(HEAD_JUNK, "h")
    h0 = spool.tile([P, 2 * QC], f32, tag="s", name="h0")
    for k in range(KT_TILES):
        nc.tensor.matmul(
            h0[:, 0:QC],
            wq_sb[:, k, 0:P],
            x_sb[:, k, 0:QC],
            start=(k == 0), stop=(k == KT_TILES - 1),
        )
    nc.vector.tensor_copy(qt_sb[0][:, 0:QC], h0[:, 0:QC])

    # ---- main loop: 8 generations x 16 exp slots ----
    # s matmuls are emitted ONE slot ahead of their exp so the exp
    # stream never eats the s-completion sem latency.
    s_tiles = {}

    def emit_s(g2, mt2):
        qc2, p2 = divmod(g2, 2)
        s_t = spool.tile([P, 2 * QC], f32, tag="s", name=f"s{g2}_{mt2}")
        for hh in range(2):
            nc.tensor.matmul(
                s_t[:, hh * QC:(hh + 1) * QC],
                kt_sb[p2][hh * D:(hh + 1) * D, mt2 * P:(mt2 + 1) * P],
                qt_sb[p2][hh * D:(hh + 1) * D, qc2 * QC:(qc2 + 1) * QC],
                start=True, stop=True,
            )
        s_tiles[(g2, mt2)] = s_t

    h1 = spool.tile([P, 2 * QC], f32, tag="s", name="h1")
    for k in range(KT_TILES):
        nc.tensor.matmul(
            h1[:, 0:P],
            wk_sb[:, k, 0:P],
            ctx_sb[:, k, 0:P],
            start=(k == 0), stop=(k == KT_TILES - 1),
        )
    nc.scalar.copy(kt_sb[0][:, 0:P], h1[:, 0:P])
    emit_s(0, 0)
    for g in range(NGEN):
        cur_gen[0] = g
        qc, p = divmod(g, 2)
        for mt in range(MT):
            spent = 0
            if g == 0:
                spent += drain_units(0, min(mt + 1, MT - 1))
            elif g == 1:
                spent += drain_units(2, min(mt + 1, MT - 1))
            if g >= 2 and mt == 8:
                spent += drain_thru(O_CLS[g - 2])
            es_t = es_pool.tile([P, 2 * QC], bf16, tag="es", name=f"es{g}_{mt}")
            nc.scalar.activation(
                es_t[:], s_tiles.pop((g, mt)),
                mybir.ActivationFunctionType.Exp, scale=SCALE,
            )
            es_tiles[(g, mt)] = es_t
            if mt < MT - 1:
                emit_s(g, mt + 1)
            elif g + 1 < NGEN:
                if g + 1 == 1:
                    spent += drain_units(2, 0)
                elif g + 1 >= 2:
                    spent += drain_thru(QT_CLS[divmod(g + 1, 2)])
                emit_s(g + 1, 0)
            spent += 2 * QC
            pump(SLOT_BUDGET - spent)

    # ---- tail: O(gen 7) + output projection for qc3 ----
    # All chunks first (PE runs back-to-back), then the fins: keeps
    # DVE round-trips off the in-order PE queue's critical path. Tail
    # stage copies go to ACT (idle after the last exp).
    cur_gen[0] = NGEN
    drain_thru(20)
    g7 = NGEN - 1

    def tail_fins(nt_l):
        # tail fins use the (now idle) s-pool banks; stage copies split
        # across ACT (ec0) and DVE (ec1), both idle after the last exp.
        tt = spool.tile([P, 2 * QC], f32, tag="s", name=f"tfin{nt_l}")
        fin_unit(12 + nt_l, 0, ft=tt[:, 0:QC], stage_act=True)
        fin_unit(12 + nt_l, 1, ft=tt[:, QC:2 * QC], stage_act=False)

    fins_after = {2: 0, 4: 1, 6: 2}
    for c in range(8):
        o_chunk(g7, c)
        if c in fins_after:
            tail_fins(fins_after[c])
    o_drain(g7, 7)
    tail_fins(3)


def _build(reps=1, opt=None):
    key = (reps, tuple(sorted((opt or {}).items())))
    if key in _CACHE:
        return _CACHE[key]
    nc = bacc.Bacc("TRN2", target_bir_lowering=False, debug=False)
    xd = nc.dram_tensor("xd", [P, KT_TILES, N], bf16, kind="ExternalInput")
    ctxd = nc.dram_tensor("ctxd", [P, KT_TILES, M], bf16, kind="ExternalInput")
    wq = nc.dram_tensor("wq", [P, KT_TILES, IC], bf16, kind="ExternalInput")
    wk = nc.dram_tensor("wk", [P, KT_TILES, IC], bf16, kind="ExternalInput")
    wv = nc.dram_tensor("wv", [P, KT_TILES, IC], bf16, kind="ExternalInput")
    wo = nc.dram_tensor("wo", [P, 2, C], bf16, kind="ExternalInput")
    identd = nc.dram_tensor("ident", [P, P], bf16, kind="ExternalInput")
    out = nc.dram_tensor("out", [N, C], bf16, kind="ExternalOutput")
    with tile.TileContext(nc) as tc:
        for _ in range(reps):
            with ExitStack() as ctx:
                _body(nc, tc, ctx, xd, ctxd, wq, wk, wv, wo, identd, out,
                      opt=opt)
    nc.compile()
    _CACHE[key] = nc
    return nc


def _to_tiled(a, inner):
    """[K*128, inner] f32 -> [128, K, inner] bf16 (partition-major tiling)."""
    k = a.shape[0] // P
    return np.ascontiguousarray(
        a.reshape(k, P, inner).transpose(1, 0, 2).astype(ml_dtypes.bfloat16)
    )


def _shard_inputs(x, context, Wq, Wk, Wv, Wo):
    ident = np.eye(P, dtype=ml_dtypes.bfloat16)
    in_maps = []
    for c in range(NCORES):
        b, g = divmod(c, NCORES // B)
        cols = slice(g * IC, (g + 1) * IC)
        in_maps.append({
            "xd": _to_tiled(np.ascontiguousarray(x[b].T), N),
            "ctxd": _to_tiled(np.ascontiguousarray(context[b].T), M),
            "wq": _to_tiled(np.ascontiguousarray(Wq[:, cols]), IC),
            "wk": _to_tiled(np.ascontiguousarray(Wk[:, cols]), IC),
            "wv": _to_tiled(np.ascontiguousarray(Wv[:, cols]), IC),
            "wo": _to_tiled(np.ascontiguousarray(Wo[cols, :]), C),
            "ident": ident,
        })
    return in_maps


def kernel(x, context, Wq, Wk, Wv, Wo, reps=1):
    x = np.asarray(x, dtype=np.float32)
    context = np.asarray(context, dtype=np.float32)
    Wq, Wk, Wv, Wo = (np.asarray(w, dtype=np.float32) for w in (Wq, Wk, Wv, Wo))
    nc = _build(reps)
    in_maps = _shard_inputs(x, context, Wq, Wk, Wv, Wo)
    res = run_bass_kernel_spmd(nc, in_maps, core_ids=list(range(NCORES)))
    gpb = NCORES // B
    out = np.zeros((B, N, C), dtype=np.float32)
    for c in range(NCORES):
        out[c // gpb] += np.asarray(res.results[c]["out"], dtype=np.float32)
    return out
